# revision 19
# baseline (speedup 1.0000x reference)
import sys

sys.path.insert(0, "/opt/trn_rl_repo")

import ml_dtypes
import numpy as np

from concourse import bacc, bass, tile
from concourse.bass_utils import run_bass_kernel_spmd

# Full problem: out[m, n] = sum_{r,k} A[r, m, k] * W[r, n, k]
#   A: [8, 8192, 256], W: [8, 2048, 256]  ->  out: [8192, 2048], viewed [8, 1024, 2048].
# Sharding: data-parallel over M. Core r computes output rows [r*1024, (r+1)*1024)
# from its A row-block (all K) plus the full weight — no collective needed.
WORLD, M, N, K = 8, 8192, 2048, 2048
M_PER = M // WORLD  # 1024
KC = K // 128  # 16 k-chunks of 128
NB = N // 512  # 4 n-chunks of 512
MT = M_PER // 128  # 8 m-tiles of 128
HEAD = 2  # m-tiles computed while the load streams (8 PSUM banks / 4 each)
F32 = bass.mybir.dt.float32
BF16 = bass.mybir.dt.bfloat16

_NC_CACHE = {}


def _build():
    if "nc" in _NC_CACHE:
        return _NC_CACHE["nc"]
    nc = bacc.Bacc(None, target_bir_lowering=False, num_devices=WORLD)
    # DRAM layouts are pre-swizzled on host so every DMA line is contiguous:
    #   a_t[p, kc*1024 + m] = A_rows^T[kc*128 + p, m]   (k on partitions)
    #   w_t[p, kc*2048 + n] = W^T[kc*128 + p, n]
    AT = nc.dram_tensor("a_t", [128, KC * M_PER], BF16, kind="ExternalInput")
    WT = nc.dram_tensor("w_t", [128, KC * N], BF16, kind="ExternalInput")
    out = nc.dram_tensor("out", [M_PER, N], F32, kind="ExternalOutput")

    MH = HEAD * 128  # head columns of each A chunk

    with tile.TileContext(nc) as tc:
        with (
            tc.tile_pool(name="wp", bufs=1) as wp,
            tc.tile_pool(name="ap", bufs=1) as ap,
            tc.tile_pool(name="rp", bufs=8) as rp,
            tc.tile_pool(name="sp", bufs=1) as sp,
            tc.tile_pool(name="ps", bufs=8, space="PSUM") as ps,
        ):
            Wt = wp.tile([128, KC, N], BF16)  # 64 KB/partition
            At = ap.tile([128, KC, M_PER], BF16)  # 32 KB/partition
            scratch = sp.tile([128, 512], BF16)

            # PE pre-warm: dummy matmuls on a memset scratch tile keep the PE
            # HAM activity window busy (~4us) during the NEFF preamble and
            # first-chunk DMA wait, so real matmuls start at 2.4 GHz. The
            nc.gpsimd.memset(scratch[:], 0)
            warm = ps.tile([128, 512], F32, name="warm", tag="acc")
            for _ in range(10):
                nc.tensor.matmul(
                    warm[:], scratch[:, 0:128], scratch[:], start=True, stop=True
                )

            # Load schedule tuned to PE consumption order: per k-chunk, the
            # head m-tiles' A slice (64KB) + the full W chunk (512KB) stream
            # first — arrival cadence (~1.6us/chunk) stays ahead of the two
            # in-flight head tiles' matmul rate (~1.7us/chunk). The remaining
            # A columns follow; they land long before the dense phase needs
            # them.
            for kc in range(KC):
                nc.sync.dma_start(
                    At[:, kc, 0:MH], AT[:, kc * M_PER : kc * M_PER + MH]
                )
                nc.sync.dma_start(Wt[:, kc, :], WT[:, kc * N : (kc + 1) * N])
            for kc in range(KC):
                nc.sync.dma_start(
                    At[:, kc, MH:M_PER], AT[:, kc * M_PER + MH : (kc + 1) * M_PER]
                )

            def alloc_accs(mi):
                return [
                    ps.tile([128, 512], F32, name=f"acc{mi}_{ni}", tag="acc")
                    for ni in range(NB)
                ]

            def mm_group(accs, mi, kc):
                lhsT = At[:, kc, mi * 128 : (mi + 1) * 128]
                for ni in range(NB):
                    nc.tensor.matmul(
                        accs[ni][:],
                        lhsT,
                        Wt[:, kc, ni * 512 : (ni + 1) * 512],
                        start=(kc == 0),
                        stop=(kc == KC - 1),
                    )

            def drain(accs, mi, fine):
                if fine:
                    # fine-grained tail: store each 512-col slice as soon as
                    # its copy lands, alternating HWDGE rings so the store
                    # completion-semaphore chains run in parallel.
                    for ni in range(NB):
                        sl = slice(ni * 512, (ni + 1) * 512)
                        row = rp.tile([128, 512], F32, name=f"rowf{mi}{ni}", tag="rf")
                        nc.vector.tensor_copy(row[:], accs[ni][:])
                        eng = nc.sync if ni % 2 == 0 else nc.scalar
                        eng.dma_start(out[mi * 128 : (mi + 1) * 128, sl], row[:])
                else:
                    row = rp.tile([128, N], F32, name=f"row{mi}", tag="row")
                    for ni in range(NB):
                        nc.vector.tensor_copy(
                            row[:, ni * 512 : (ni + 1) * 512], accs[ni][:]
                        )
                    nc.scalar.dma_start(out[mi * 128 : (mi + 1) * 128, :], row[:])

            # Head m-tiles walk k in DMA-arrival order (8 PSUM banks =
            # 2 concurrent m-tiles), hiding compute under the load phase.
            head = {mi: alloc_accs(mi) for mi in range(HEAD)}
            for kc in range(KC):
                for mi in range(HEAD):
                    mm_group(head[mi], mi, kc)
            for mi in range(HEAD):
                drain(head[mi], mi, fine=True)

            # Remaining m-tiles run dense (all chunks resident by now).
            for mi in range(HEAD, MT):
                accs = alloc_accs(mi)
                for kc in range(KC):
                    mm_group(accs, mi, kc)
                drain(accs, mi, fine=True)
    nc.compile()
    _NC_CACHE["nc"] = nc
    return nc


def _shard_inputs(A, weight):
    A = np.ascontiguousarray(np.asarray(A), dtype=np.float32)
    weight = np.ascontiguousarray(np.asarray(weight), dtype=np.float32)
    # W^T: [K, N] with k = r*256 + k_local; then swizzle to [128, KC*N]
    wt = weight.transpose(0, 2, 1).reshape(K, N)
    wt = (
        wt.reshape(KC, 128, N)
        .transpose(1, 0, 2)
        .reshape(128, KC * N)
        .astype(ml_dtypes.bfloat16)
    )
    in_maps = []
    for r in range(WORLD):
        at = A[:, r * M_PER : (r + 1) * M_PER, :].transpose(0, 2, 1).reshape(K, M_PER)
        at = (
            at.reshape(KC, 128, M_PER)
            .transpose(1, 0, 2)
            .reshape(128, KC * M_PER)
            .astype(ml_dtypes.bfloat16)
        )
        in_maps.append({"a_t": np.ascontiguousarray(at), "w_t": wt})
    return in_maps


def _run(A, weight, trace=False):
    nc = _build()
    in_maps = _shard_inputs(A, weight)
    res = run_bass_kernel_spmd(nc, in_maps, core_ids=list(range(WORLD)), trace=trace)
    out = np.stack([res.results[r]["out"] for r in range(WORLD)], axis=0)
    return out, res


def kernel(A, weight):
    out, _ = _run(A, weight)
    return out


# revision 20
# speedup vs baseline: 1.0084x; 1.0084x over previous
import sys

sys.path.insert(0, "/opt/trn_rl_repo")

import ml_dtypes
import numpy as np

from concourse import bacc, bass, tile
from concourse.bass_utils import run_bass_kernel_spmd

# Full problem: out[m, n] = sum_{r,k} A[r, m, k] * W[r, n, k]
#   A: [8, 8192, 256], W: [8, 2048, 256]  ->  out: [8192, 2048], viewed [8, 1024, 2048].
# Sharding: data-parallel over M. Core r computes output rows [r*1024, (r+1)*1024)
# from its A row-block (all K) plus the full weight — no collective needed.
WORLD, M, N, K = 8, 8192, 2048, 2048
M_PER = M // WORLD  # 1024
KC = K // 128  # 16 k-chunks of 128
NB = N // 512  # 4 n-chunks of 512
MT = M_PER // 128  # 8 m-tiles of 128
HEAD = 2  # m-tiles computed while the load streams (8 PSUM banks / 4 each)
F32 = bass.mybir.dt.float32
BF16 = bass.mybir.dt.bfloat16

_NC_CACHE = {}


def _build():
    if "nc" in _NC_CACHE:
        return _NC_CACHE["nc"]
    nc = bacc.Bacc(None, target_bir_lowering=False, num_devices=WORLD)
    # DRAM layouts are pre-swizzled on host so every DMA line is contiguous:
    #   a_t[p, kc*1024 + m] = A_rows^T[kc*128 + p, m]   (k on partitions)
    #   w_t[p, kc*2048 + n] = W^T[kc*128 + p, n]
    AT = nc.dram_tensor("a_t", [128, KC * M_PER], BF16, kind="ExternalInput")
    WT = nc.dram_tensor("w_t", [128, KC * N], BF16, kind="ExternalInput")
    out = nc.dram_tensor("out", [M_PER, N], F32, kind="ExternalOutput")

    MH = HEAD * 128  # head columns of each A chunk

    with tile.TileContext(nc) as tc:
        with (
            tc.tile_pool(name="wp", bufs=1) as wp,
            tc.tile_pool(name="ap", bufs=1) as ap,
            tc.tile_pool(name="rp", bufs=4) as rp,
            tc.tile_pool(name="sp", bufs=1) as sp,
            tc.tile_pool(name="ps", bufs=8, space="PSUM") as ps,
        ):
            Wt = wp.tile([128, KC, N], BF16)  # 64 KB/partition
            At = ap.tile([128, KC, M_PER], BF16)  # 32 KB/partition
            scratch = sp.tile([128, 512], BF16)

            # PE pre-warm: dummy matmuls on a memset scratch tile keep the PE
            # HAM activity window busy (~4us) during the NEFF preamble and
            # first-chunk DMA wait, so real matmuls start at 2.4 GHz. The
            nc.gpsimd.memset(scratch[:], 0)
            warm = ps.tile([128, 512], F32, name="warm", tag="acc")
            for _ in range(10):
                nc.tensor.matmul(
                    warm[:], scratch[:, 0:128], scratch[:], start=True, stop=True
                )

            # Load schedule tuned to PE consumption order: per k-chunk, the
            # head m-tiles' A slice (64KB) + the full W chunk (512KB) stream
            # first — arrival cadence (~1.6us/chunk) stays ahead of the two
            # in-flight head tiles' matmul rate (~1.7us/chunk). The remaining
            # A columns follow; they land long before the dense phase needs
            # them.
            for kc in range(KC):
                nc.sync.dma_start(
                    At[:, kc, 0:MH], AT[:, kc * M_PER : kc * M_PER + MH]
                )
                nc.sync.dma_start(Wt[:, kc, :], WT[:, kc * N : (kc + 1) * N])
            for kc in range(KC):
                nc.sync.dma_start(
                    At[:, kc, MH:M_PER], AT[:, kc * M_PER + MH : (kc + 1) * M_PER]
                )

            def alloc_accs(mi):
                return [
                    ps.tile([128, 512], F32, name=f"acc{mi}_{ni}", tag="acc")
                    for ni in range(NB)
                ]

            def mm_group(accs, mi, kc):
                lhsT = At[:, kc, mi * 128 : (mi + 1) * 128]
                for ni in range(NB):
                    nc.tensor.matmul(
                        accs[ni][:],
                        lhsT,
                        Wt[:, kc, ni * 512 : (ni + 1) * 512],
                        start=(kc == 0),
                        stop=(kc == KC - 1),
                    )

            def drain(accs, mi, fine):
                if fine:
                    # fine-grained tail: store each 512-col slice as soon as
                    # its copy lands, alternating HWDGE rings so the store
                    # completion-semaphore chains run in parallel.
                    for ni in range(NB):
                        sl = slice(ni * 512, (ni + 1) * 512)
                        row = rp.tile([128, 512], F32, name=f"rowf{mi}{ni}", tag="rf")
                        nc.vector.tensor_copy(row[:], accs[ni][:])
                        eng = nc.sync if ni % 2 == 0 else nc.scalar
                        eng.dma_start(out[mi * 128 : (mi + 1) * 128, sl], row[:])
                else:
                    row = rp.tile([128, N], F32, name=f"row{mi}", tag="row")
                    for ni in range(NB):
                        nc.vector.tensor_copy(
                            row[:, ni * 512 : (ni + 1) * 512], accs[ni][:]
                        )
                    nc.scalar.dma_start(out[mi * 128 : (mi + 1) * 128, :], row[:])

            # Head m-tiles walk k in DMA-arrival order (8 PSUM banks =
            # 2 concurrent m-tiles), hiding compute under the load phase.
            head = {mi: alloc_accs(mi) for mi in range(HEAD)}
            for kc in range(KC):
                for mi in range(HEAD):
                    mm_group(head[mi], mi, kc)
            for mi in range(HEAD):
                drain(head[mi], mi, fine=True)

            # Remaining m-tiles run dense (all chunks resident by now).
            for mi in range(HEAD, MT):
                accs = alloc_accs(mi)
                for kc in range(KC):
                    mm_group(accs, mi, kc)
                drain(accs, mi, fine=True)
    nc.compile()
    _NC_CACHE["nc"] = nc
    return nc


def _shard_inputs(A, weight):
    A = np.ascontiguousarray(np.asarray(A), dtype=np.float32)
    weight = np.ascontiguousarray(np.asarray(weight), dtype=np.float32)
    # W^T: [K, N] with k = r*256 + k_local; then swizzle to [128, KC*N]
    wt = weight.transpose(0, 2, 1).reshape(K, N)
    wt = (
        wt.reshape(KC, 128, N)
        .transpose(1, 0, 2)
        .reshape(128, KC * N)
        .astype(ml_dtypes.bfloat16)
    )
    in_maps = []
    for r in range(WORLD):
        at = A[:, r * M_PER : (r + 1) * M_PER, :].transpose(0, 2, 1).reshape(K, M_PER)
        at = (
            at.reshape(KC, 128, M_PER)
            .transpose(1, 0, 2)
            .reshape(128, KC * M_PER)
            .astype(ml_dtypes.bfloat16)
        )
        in_maps.append({"a_t": np.ascontiguousarray(at), "w_t": wt})
    return in_maps


def _run(A, weight, trace=False):
    nc = _build()
    in_maps = _shard_inputs(A, weight)
    res = run_bass_kernel_spmd(nc, in_maps, core_ids=list(range(WORLD)), trace=trace)
    out = np.stack([res.results[r]["out"] for r in range(WORLD)], axis=0)
    return out, res


def kernel(A, weight):
    out, _ = _run(A, weight)
    return out
